# revision 21
# baseline (speedup 1.0000x reference)
"""GQA kernel for trn2, 8 NeuronCores.

Problem: nn_GroupedQueryAttention (b=4, s=2048, 16 q-heads / 4 kv-heads, d=64).
The reference's score einsum 'bghsd,bhad->bhsa' SUMS over the group axis g, and
RoPE is linear in x, so sum_g rope(q @ Wq[:,h*4+g,:]) == rope(q @ sum_g Wq).
The whole module therefore collapses to 4-head MHA with Wq pre-summed over
groups.

Sharding: 8 cores = (batch b in 0..3) x (head-group hg in 0..1, 2 heads each).
Each core computes its two heads' attention output and a partial product with
its 128-row slice of Wo; the host sums the two partials per batch.

Device layout choices (all host-prepped in numpy):
 - qT [1024, 2048] (q[b] transposed, bf16) so the contraction dim i sits on
   SBUF partitions for every projection matmul.
 - q/k projection weights are group-summed, and their 64 head-dims are
   de-interleaved to [32 real | 32 imag] so RoPE becomes
     roped = x * CC + xswap * SS
   where xswap is produced by a second matmul with column-swapped weights.
 - all matmul operands are bf16 (1 PE cycle/row vs 4 for fp32); PSUM
   accumulation stays fp32.
 - scores are computed TRANSPOSED (scoresT[a, s]) so no on-device transpose of
   the attention matrix is needed; softmax denominators come from a ones-column
   appended to V (the attn@V matmul computes them for free along M).
 - causal masking: additive -1e4 bias, injected into the scores PSUM
   accumulation group via an identity-matmul on the 4 diagonal-crossing tiles;
   score/exp/attnV streams on those tiles skip the fully-masked column prefix.
 - softmax 1/rowsum is broadcast across the 64 head dims with a tiny K=1
   PE matmul (ones[1,64] x rec[1,512]).

Scheduling: one fused loop over the 4 s-chunks. Chunk j emits
  qproj(j) -> kproj(j) -> vproj(tiles 4j..4j+3) -> attention(j)
and attention is software-pipelined two heads wide with one-tile lookahead
(score matmuls for tile t+1 are issued before attnV of tile t) so the PE
never head-of-line blocks on the Act engine's exp. The Wo output projection
for chunk j-1 is emitted inside chunk j's attention loop, overlapping the
output DMA with compute.
"""

import numpy as np

B, S, IN_DIM = 4, 2048, 1024
Q_HEADS, KV_HEADS, HEAD_DIM = 16, 4, 64
GROUPS = Q_HEADS // KV_HEADS
HALF = HEAD_DIM // 2  # 32
N_CORES = 8
SC = 512  # s-chunk width (psum bank)
AT = 128  # a-tile width
MASK_BIAS = -1.0e4

_cached = {}


def _install_wait_splitter():
    """This walrus build accepts only ONE semaphore wait per instruction.
    Tile emits several; hoist all-but-one into standalone EventSemaphores."""
    import concourse.mybir as mybir
    import concourse.tile as tile
    from concourse._compat import not_none as nn

    if getattr(tile.TileContext, "_wait_split_installed", False):
        return

    orig_add = tile.TileContext._add_instruction

    def patched_add(self, inst):
        si = getattr(inst, "sync_info", None)
        if si is not None and si.on_wait and len(si.on_wait) > 1:
            waits = list(si.on_wait)
            for w in waits[:-1]:
                nm = self.nc.get_next_instruction_name()
                ev = mybir.InstEventSemaphore(
                    name=nm, engine=inst.engine, ins=[], outs=[],
                    sync_info=mybir.SyncInfo(on_wait=[w], on_update=[]))
                orig_add(self, ev)
            inst.sync_info = mybir.SyncInfo(
                on_wait=[waits[-1]], on_update=list(si.on_update or []))
        orig_add(self, inst)

    def patched_drain(self, tick_clock, wait_clock):
        # reimplementation of the original: same drain -> barrier -> sem-clear
        # -> barrier sequence, but the drain's (many) waits are split into
        # standalone EventSemaphores emitted BEFORE the sem clear.
        from concourse.vector_clock import ScopedClock

        nc = self.nc
        drain_wrap = nc.sync.drain()
        drain_inst = drain_wrap.ins  # BassInstruction wrapper -> mybir inst
        wait_clock.add_sem_waits(
            drain_inst, ScopedClock({None: tick_clock.global_clock}))
        bb = nn(nc.cur_bb).bb
        si = getattr(drain_inst, "sync_info", None)
        if si is not None and si.on_wait and len(si.on_wait) > 1:
            waits = list(si.on_wait)
            drain_inst.sync_info = mybir.SyncInfo(
                on_wait=[waits[0]], on_update=list(si.on_update or []))
            for w in waits[1:]:
                nm = nc.get_next_instruction_name()
                ev = mybir.InstEventSemaphore(
                    name=nm, engine=drain_inst.engine, ins=[], outs=[],
                    sync_info=mybir.SyncInfo(on_wait=[w], on_update=[]))
                nc.register_instruction(ev, overwrite=True)
                bb.add_instruction(ev)

        nc.all_engine_barrier()
        assert self.sems is not None
        popped = nc._tile_sem_poison_stack.pop()
        assert popped is self._sem_poison
        nc.clear_and_free_semaphores(list(self.sems.allocated().values()))
        nc.all_engine_barrier()

    tile.TileContext._add_instruction = patched_add
    tile.TileContext._drain_and_barrier = patched_drain
    tile.TileContext._wait_split_installed = True


def _build_nc():
    import concourse.bass as bass
    import concourse.mybir as mybir
    import concourse.tile as tile

    _install_wait_splitter()

    f32 = mybir.dt.float32
    bf16 = mybir.dt.bfloat16

    nc = bass.Bass()

    qT = nc.declare_dram_parameter("qT", [IN_DIM, S], bf16, isOutput=False)
    wq = nc.declare_dram_parameter("wq", [IN_DIM, 128], bf16, isOutput=False)
    wk = nc.declare_dram_parameter("wk", [IN_DIM, 128], bf16, isOutput=False)
    wv = nc.declare_dram_parameter("wv", [IN_DIM, 128], bf16, isOutput=False)
    wo = nc.declare_dram_parameter("wo", [128, IN_DIM], bf16, isOutput=False)
    cc = nc.declare_dram_parameter("cc", [128, S], bf16, isOutput=False)
    ssp = nc.declare_dram_parameter("ssp", [128, S], bf16, isOutput=False)
    mb = nc.declare_dram_parameter("mb", [128, 4 * SC], bf16, isOutput=False)
    ident = nc.declare_dram_parameter("ident", [128, 128], bf16, isOutput=False)
    perm = nc.declare_dram_parameter("perm", [128, 128], bf16, isOutput=False)
    out = nc.declare_dram_parameter("out", [S, IN_DIM], bf16, isOutput=True)

    NSC = S // SC   # 4 s-chunks
    NAT = S // AT   # 16 a-tiles
    NIT = IN_DIM // 128  # 8 i-tiles
    EXP_SCALE = float(1.0 / np.sqrt(HEAD_DIM))

    with tile.TileContext(nc) as tc:
        with (
            tc.tile_pool(name="big", bufs=1) as big,
            tc.tile_pool(name="psum", bufs=4, space="PSUM") as psum,
            tc.tile_pool(name="tmp", bufs=3) as tmp,
            tc.tile_pool(name="attn", bufs=6) as attnp,
            tc.tile_pool(name="small", bufs=2) as small,
            tc.tile_pool(name="osb", bufs=2) as osb,
        ):
            # ---- resident SBUF tensors ----
            qT_sb = big.tile([128, NIT, S], bf16)
            wq_sb = big.tile([128, NIT, 128], bf16)
            wk_sb = big.tile([128, NIT, 128], bf16)
            wv_sb = big.tile([128, NIT, 128], bf16)
            wo_sb = big.tile([128, IN_DIM], bf16)
            cc_sb = big.tile([128, S], bf16)
            ssp_sb = big.tile([128, S], bf16)
            mb_sb = big.tile([128, 4 * SC], bf16)
            id_sb = big.tile([128, 128], bf16)
            pm_sb = big.tile([128, 128], bf16)
            qh_sb = big.tile([128, S], bf16)  # roped q, [2 heads x (32r|32i)], s
            kh_sb = big.tile([128, S], bf16)
            v_sb = big.tile([128, 2, NAT, HEAD_DIM + 1], bf16)  # [a, h, t, d+1]
            on_sb = big.tile([128, S], bf16)  # normalized outT, 2 heads stacked
            ones_sb = big.tile([1, HEAD_DIM], bf16)

            # DMA emission order = first-need order, every transfer split to
            # <=128KB pieces so the 16 queues deliver the first-needed bytes
            # in parallel (a queue moves only ~20GB/s; one big transfer on
            # one queue serializes the whole warmup).
            def dma_w_tiles(w_sb, w, tiles):
                for t in tiles:
                    nc.sync.dma_start(out=w_sb[:, t, :],
                                      in_=w[t * 128:(t + 1) * 128, :])

            def dma_qt_chunk(c, halves=False):
                cs = slice(c * SC, (c + 1) * SC)
                for t in range(NIT):
                    if halves:
                        m = c * SC + SC // 2
                        nc.sync.dma_start(out=qT_sb[:, t, c * SC:m],
                                          in_=qT[t * 128:(t + 1) * 128,
                                                 c * SC:m])
                        nc.sync.dma_start(out=qT_sb[:, t, m:(c + 1) * SC],
                                          in_=qT[t * 128:(t + 1) * 128,
                                                 m:(c + 1) * SC])
                    else:
                        nc.sync.dma_start(out=qT_sb[:, t, cs],
                                          in_=qT[t * 128:(t + 1) * 128, cs])
                nc.sync.dma_start(out=cc_sb[:, cs], in_=cc[:, cs])
                nc.sync.dma_start(out=ssp_sb[:, cs], in_=ssp[:, cs])

            dma_w_tiles(wq_sb, wq, range(NIT))
            dma_qt_chunk(0, halves=True)
            dma_w_tiles(wk_sb, wk, range(NIT))
            dma_w_tiles(wv_sb, wv, range(NIT))
            nc.sync.dma_start(out=pm_sb, in_=perm[:, :])
            dma_qt_chunk(1)
            for c in range(4):
                nc.sync.dma_start(out=mb_sb[:, c * SC:(c + 1) * SC],
                                  in_=mb[:, c * SC:(c + 1) * SC])
            nc.sync.dma_start(out=id_sb, in_=ident[:, :])
            dma_qt_chunk(2)
            nc.sync.dma_start(out=wo_sb[:, 0:SC], in_=wo[:, 0:SC])
            nc.sync.dma_start(out=wo_sb[:, SC:IN_DIM], in_=wo[:, SC:IN_DIM])
            dma_qt_chunk(3)
            nc.vector.memset(v_sb, 1.0)  # ones column for rowsums survives
            nc.vector.memset(ones_sb, 1.0)

            def emit_qk_finish_dve(j, accs):
                """rope partial products: t1 = x*cc (f32), t2p = x*ssP (bf16,
                feeds the partial-swap permutation matmul)."""
                cs = slice(j * SC, (j + 1) * SC)
                parts = []
                for ps_x in accs:
                    t2p = tmp.tile([128, SC], bf16, tag="t2p", name="t2p")
                    nc.vector.tensor_mul(t2p, ps_x, ssp_sb[:, cs])
                    t1 = tmp.tile([128, SC], f32, tag="t1", name="t1")
                    nc.vector.tensor_mul(t1, ps_x, cc_sb[:, cs])
                    parts.append((t1, t2p))
                return parts

            def emit_qk_finish_pe(j, parts):
                """swap halves of the ssP product via a permutation matmul,
                then combine: dst = x*cc + P(x*ssP)."""
                cs = slice(j * SC, (j + 1) * SC)
                for (t1, t2p), dst in zip(parts, (qh_sb, kh_sb)):
                    ps_t2 = psum.tile([128, SC], f32, tag="ps", name="ps_t2")
                    nc.tensor.matmul(ps_t2, pm_sb, t2p, start=True, stop=True)
                    nc.vector.tensor_add(dst[:, cs], t1, ps_t2)

            def emit_v_tile(t):
                ps_v = psum.tile([128, SC], f32, tag="ps", name="ps_v")
                for ti in range(NIT):
                    nc.tensor.matmul(
                        ps_v[:, 0:128],
                        qT_sb[:, ti, t * AT:(t + 1) * AT], wv_sb[:, ti, :],
                        start=(ti == 0), stop=(ti == NIT - 1))
                nc.vector.tensor_copy(
                    v_sb[:, :, t, 0:HEAD_DIM],
                    ps_v[:, 0:128].rearrange("p (h d) -> p h d", h=2))

            def emit_score(h, j, t, n_at):
                """scores matmul (+mask) for a-tile t of chunk j, head h,
                then exp into a bf16 attn tile. Returns (at, off)."""
                hp = slice(64 * h, 64 * h + 64)
                diag = t >= n_at - 4
                off = (t - 4 * j) * AT if diag else 0
                js = slice(j * SC + off, (j + 1) * SC)
                ps_s = psum.tile([128, SC], f32, tag="ps", name="ps_s")
                nc.tensor.matmul(
                    ps_s[:, off:SC],
                    kh_sb[hp, t * AT:(t + 1) * AT], qh_sb[hp, js],
                    start=True, stop=not diag)
                if diag:
                    c = t - 4 * j
                    nc.tensor.matmul(
                        ps_s[:, off:SC], id_sb,
                        mb_sb[:, c * SC + off:(c + 1) * SC],
                        start=False, stop=True)
                at = attnp.tile([128, SC], bf16, tag="at", name="at")
                nc.scalar.activation(
                    out=at[:, off:SC], in_=ps_s[:, off:SC],
                    func=mybir.ActivationFunctionType.Exp, scale=EXP_SCALE)
                return at, off

            def emit_attnv(h, t, n_at, ps_o, at, off):
                nc.tensor.matmul(
                    ps_o[0:HEAD_DIM + 1, off:SC],
                    v_sb[:, h, t, :], at[:, off:SC],
                    start=(t == 0), stop=(t == n_at - 1),
                    skip_group_check=True)

            def emit_rowsum(ps_o):
                """1/rowsum as exp(-ln(rowsum)), both on Act: no slow DVE
                reciprocal, and both funcs live in the natural_log_exp table
                so there is no activation-table reload."""
                ln_r = small.tile([1, SC], f32, tag="lnr", name="ln_r")
                nc.scalar.activation(
                    out=ln_r, in_=ps_o[HEAD_DIM:HEAD_DIM + 1, :],
                    func=mybir.ActivationFunctionType.Ln)
                rec_b = small.tile([1, SC], bf16, tag="recb", name="rec_b")
                nc.scalar.activation(
                    out=rec_b, in_=ln_r,
                    func=mybir.ActivationFunctionType.Exp, scale=-1.0)
                return rec_b

            def emit_normalize(h, j, ps_o, rec_b):
                hp = slice(64 * h, 64 * h + 64)
                js = slice(j * SC, (j + 1) * SC)
                ps_r = psum.tile([HEAD_DIM, SC], f32, tag="ps", name="ps_r")
                nc.tensor.matmul(ps_r, ones_sb, rec_b, start=True, stop=True)
                sb_r = tmp.tile([HEAD_DIM, SC], f32, tag="sbr", name="sb_r")
                nc.vector.tensor_copy(sb_r, ps_r)
                tn = tmp.tile([64, SC], bf16, tag="tn", name="tn")
                nc.vector.tensor_mul(tn, ps_o[0:HEAD_DIM, :], sb_r)
                nc.sync.dma_start(out=on_sb[hp, js], in_=tn)

            def emit_wo_m(m):
                o_sb = osb.tile([128, IN_DIM], bf16, tag="o", name="o_sb")
                for c in range(IN_DIM // SC):
                    ps = psum.tile([128, SC], f32, tag="ps", name="ps_w")
                    nc.tensor.matmul(
                        ps, on_sb[:, m * 128:(m + 1) * 128],
                        wo_sb[:, c * SC:(c + 1) * SC],
                        start=True, stop=True)
                    nc.vector.tensor_copy(o_sb[:, c * SC:(c + 1) * SC], ps)
                nc.sync.dma_start(out=out[m * 128:(m + 1) * 128, :], in_=o_sb)

            def alloc_accs(i):
                return [psum.tile([128, SC], f32, tag="pa", bufs=2,
                                  name=f"ps_x{i}{k}") for k in range(2)]

            def emit_accum_half(j, accs, half):
                cs = slice(j * SC, (j + 1) * SC)
                for t in range(half * (NIT // 2), (half + 1) * (NIT // 2)):
                    for w_sb, ps_x in zip((wq_sb, wk_sb), accs):
                        nc.tensor.matmul(
                            ps_x, w_sb[:, t, :], qT_sb[:, t, cs],
                            start=(t == 0), stop=(t == NIT - 1))

            # ---- fused chunk loop, fully software-pipelined ----
            # Each attention iteration is Act-gated (2 exps ~ 1.15us vs
            # ~0.9us of its own matmuls), so every other chunk's PE work
            # (v proj, next-next chunk's qk accum, previous chunk's Wo) is
            # queued as filler thunks consumed one per iteration: the PE
            # stays continuously busy (holding its fast p-state) and all
            # cross-engine chains get covered by independent queued work.
            accs = {0: alloc_accs(0)}
            emit_accum_half(0, accs[0], 0)
            emit_accum_half(0, accs[0], 1)
            parts1 = emit_qk_finish_dve(0, accs[0])
            emit_qk_finish_pe(0, parts1)
            for t in range(4):
                emit_v_tile(t)
            accs[1] = alloc_accs(1)
            emit_accum_half(1, accs[1], 0)
            emit_accum_half(1, accs[1], 1)
            parts1 = None
            for j in range(NSC):
                n_at = 4 * (j + 1)
                queue = []
                if j > 0:
                    queue += [(emit_wo_m, (m,))
                              for m in range(4 * (j - 1), 4 * j)]
                vtail = []
                if j + 1 < NSC:
                    vt = list(range(4 * (j + 1), 4 * (j + 2)))
                    queue += [(emit_v_tile, (t,)) for t in vt[:2]]
                    vtail = vt[2:]
                if j + 2 < NSC:
                    accs[j + 2] = alloc_accs(j + 2)
                    queue += [(emit_accum_half, (j + 2, accs[j + 2], 0)),
                              (emit_accum_half, (j + 2, accs[j + 2], 1))]
                ps_o = [psum.tile([128, SC], f32, tag="po", bufs=2,
                                  name=f"ps_o{h}") for h in range(2)]
                pend = {}  # (h, t) -> (at, off)
                for h in range(2):
                    pend[(h, 0)] = emit_score(h, j, 0, n_at)
                for t in range(n_at):
                    if t + 1 < n_at:
                        for h in range(2):
                            pend[(h, t + 1)] = emit_score(h, j, t + 1, n_at)
                    for h in range(2):
                        at, off = pend.pop((h, t))
                        emit_attnv(h, t, n_at, ps_o[h], at, off)
                    if t == 0 and (j + 1) in accs:
                        parts1 = emit_qk_finish_dve(j + 1, accs[j + 1])
                    elif t == 2 and parts1 is not None:
                        emit_qk_finish_pe(j + 1, parts1)
                        parts1 = None
                    elif queue:
                        fn, args = queue.pop(0)
                        fn(*args)
                recs = [emit_rowsum(ps_o[h]) for h in range(2)]
                for fn, args in queue:
                    fn(*args)
                for t in vtail:
                    emit_v_tile(t)
                for h in range(2):
                    emit_normalize(h, j, ps_o[h], recs[h])
            for m in range(4 * (NSC - 1), 4 * NSC):
                emit_wo_m(m)

    return nc


def _host_prep(q, Wq, Wk, Wv, Wo):
    """Build the 8 per-core input maps (numpy, bf16 for matmul operands)."""
    import ml_dtypes
    bf16 = ml_dtypes.bfloat16

    deint = np.concatenate([np.arange(0, HEAD_DIM, 2),
                            np.arange(1, HEAD_DIM, 2)])  # de-interleave perm

    wq_eff = Wq.reshape(IN_DIM, KV_HEADS, GROUPS, HEAD_DIM).sum(axis=2)
    wq_d = wq_eff[:, :, deint]          # [1024, 4, 64] de-interleaved
    wk_d = Wk[:, :, deint]

    # rope tables, de-interleaved layout, one head-block repeated twice.
    # ssp is PRE-SWAPPED (rows [0:32]<->[32:64] within each 64-block): the
    # device computes P(x * ssp) with a permutation matmul, which equals
    # xswap * ss.
    pos = np.arange(1, S + 1, dtype=np.float64)
    thetas = 10.0 ** (-np.arange(HALF, dtype=np.float64))
    ang = pos[None, :] * thetas[:, None]          # [32, S]
    cosv, sinv = np.cos(ang), np.sin(ang)
    cc64 = np.concatenate([cosv, cosv], axis=0)   # [64, S]
    ssp64 = np.concatenate([sinv, -sinv], axis=0)  # P(ss), ss = [-sin|+sin]
    cc = np.concatenate([cc64, cc64], axis=0).astype(bf16)   # [128, S]
    ssp = np.concatenate([ssp64, ssp64], axis=0).astype(bf16)

    # half-swap permutation matrix (symmetric involution)
    r = np.arange(128)
    pr = (r // 64) * 64 + ((r % 64) + HALF) % 64
    perm = np.zeros((128, 128), dtype=np.float32)
    perm[pr, r] = 1.0
    perm = perm.astype(bf16)

    # additive causal bias for the 4 diagonal-crossing tiles, bf16
    i_idx = np.arange(AT)[:, None]
    j_idx = np.arange(SC)[None, :]
    mbs = []
    for c in range(4):
        delta = AT * c
        mbs.append(np.where(j_idx < i_idx + delta, MASK_BIAS, 0.0))
    mb = np.concatenate(mbs, axis=1).astype(bf16)  # [128, 2048]
    ident = np.eye(128, dtype=np.float32).astype(bf16)

    in_maps = []
    for core in range(N_CORES):
        b, hg = core // 2, core % 2
        heads = [2 * hg, 2 * hg + 1]
        wq_c = np.concatenate([wq_d[:, h, :] for h in heads], axis=1)
        wk_c = np.concatenate([wk_d[:, h, :] for h in heads], axis=1)
        wv_c = np.concatenate([Wv[:, h, :] for h in heads], axis=1)
        in_maps.append({
            "qT": np.ascontiguousarray(q[b].T).astype(bf16),
            "wq": wq_c.astype(bf16),
            "wk": wk_c.astype(bf16),
            "wv": wv_c.astype(bf16),
            "wo": Wo[hg * 128:(hg + 1) * 128, :].astype(bf16),
            "cc": cc, "ssp": ssp, "mb": mb, "ident": ident, "perm": perm,
        })
    return in_maps


def _install_ntff_hook():
    """Recreate the missing antenv.axon_hooks shim so trace=True works."""
    import sys, types
    if "antenv.axon_hooks" in sys.modules:
        return
    mod = types.ModuleType("antenv.axon_hooks")
    _hook = [None]
    mod.set_axon_ntff_profile_hook = lambda h: _hook.__setitem__(0, h)
    mod.get_axon_ntff_profile_hook = lambda: _hook[0]
    sys.modules["antenv.axon_hooks"] = mod
    try:
        if "/root/.axon_site" not in sys.path:
            sys.path.insert(0, "/root/.axon_site")
        from trn_agent_boot.trn_boot import _ntff_profile_via_ctypes
        mod.set_axon_ntff_profile_hook(
            _ntff_profile_via_ctypes("/opt/axon/libaxon_pjrt.so"))
    except Exception:
        pass


def kernel(q, mask, Wq, Wk, Wv, Wo, _trace=False):
    import sys
    if "/opt/trn_rl_repo" not in sys.path:
        sys.path.insert(0, "/opt/trn_rl_repo")
    if _trace:
        _install_ntff_hook()
    from concourse.bass_utils import run_bass_kernel_spmd

    if "nc" not in _cached:
        _cached["nc"] = _build_nc()
    nc = _cached["nc"]

    q = np.asarray(q, np.float32)
    in_maps = _host_prep(q, np.asarray(Wq, np.float32),
                         np.asarray(Wk, np.float32), np.asarray(Wv, np.float32),
                         np.asarray(Wo, np.float32))
    res = run_bass_kernel_spmd(nc, in_maps, core_ids=list(range(N_CORES)),
                               trace=_trace)
    parts = [np.asarray(r["out"], dtype=np.float32) for r in res.results]
    out = np.stack([parts[2 * b] + parts[2 * b + 1] for b in range(B)])
    if _trace:
        kernel.last_exec_time_ns = res.exec_time_ns
        kernel.last_results = res
    return out.astype(np.float32)


# revision 23
# speedup vs baseline: 1.0509x; 1.0509x over previous
"""GQA kernel for trn2, 8 NeuronCores.

Problem: nn_GroupedQueryAttention (b=4, s=2048, 16 q-heads / 4 kv-heads, d=64).
The reference's score einsum 'bghsd,bhad->bhsa' SUMS over the group axis g, and
RoPE is linear in x, so sum_g rope(q @ Wq[:,h*4+g,:]) == rope(q @ sum_g Wq).
The whole module therefore collapses to 4-head MHA with Wq pre-summed over
groups.

Sharding: 8 cores = (batch b in 0..3) x (head-group hg in 0..1, 2 heads each).
Each core computes its two heads' attention output and a partial product with
its 128-row slice of Wo; the host sums the two partials per batch.

Device layout choices (all host-prepped in numpy):
 - qT [1024, 2048] (q[b] transposed, bf16) so the contraction dim i sits on
   SBUF partitions for every projection matmul.
 - q/k projection weights are group-summed, and their 64 head-dims are
   de-interleaved to [32 real | 32 imag] so RoPE becomes
     roped = x * CC + xswap * SS
   where xswap is produced by a second matmul with column-swapped weights.
 - all matmul operands are bf16 (1 PE cycle/row vs 4 for fp32); PSUM
   accumulation stays fp32.
 - scores are computed TRANSPOSED (scoresT[a, s]) so no on-device transpose of
   the attention matrix is needed; softmax denominators come from a ones-column
   appended to V (the attn@V matmul computes them for free along M).
 - causal masking: additive -1e4 bias, injected into the scores PSUM
   accumulation group via an identity-matmul on the 4 diagonal-crossing tiles;
   score/exp/attnV streams on those tiles skip the fully-masked column prefix.
 - softmax 1/rowsum is broadcast across the 64 head dims with a tiny K=1
   PE matmul (ones[1,64] x rec[1,512]).

Scheduling: one fused loop over the 4 s-chunks. Chunk j emits
  qproj(j) -> kproj(j) -> vproj(tiles 4j..4j+3) -> attention(j)
and attention is software-pipelined two heads wide with one-tile lookahead
(score matmuls for tile t+1 are issued before attnV of tile t) so the PE
never head-of-line blocks on the Act engine's exp. The Wo output projection
for chunk j-1 is emitted inside chunk j's attention loop, overlapping the
output DMA with compute.
"""

import numpy as np

B, S, IN_DIM = 4, 2048, 1024
Q_HEADS, KV_HEADS, HEAD_DIM = 16, 4, 64
GROUPS = Q_HEADS // KV_HEADS
HALF = HEAD_DIM // 2  # 32
N_CORES = 8
SC = 512  # s-chunk width (psum bank)
AT = 128  # a-tile width
MASK_BIAS = -1.0e4

_cached = {}


def _install_wait_splitter():
    """This walrus build accepts only ONE semaphore wait per instruction.
    Tile emits several; hoist all-but-one into standalone EventSemaphores."""
    import concourse.mybir as mybir
    import concourse.tile as tile
    from concourse._compat import not_none as nn

    if getattr(tile.TileContext, "_wait_split_installed", False):
        return

    orig_add = tile.TileContext._add_instruction

    def patched_add(self, inst):
        si = getattr(inst, "sync_info", None)
        if si is not None and si.on_wait and len(si.on_wait) > 1:
            waits = list(si.on_wait)
            for w in waits[:-1]:
                nm = self.nc.get_next_instruction_name()
                ev = mybir.InstEventSemaphore(
                    name=nm, engine=inst.engine, ins=[], outs=[],
                    sync_info=mybir.SyncInfo(on_wait=[w], on_update=[]))
                orig_add(self, ev)
            inst.sync_info = mybir.SyncInfo(
                on_wait=[waits[-1]], on_update=list(si.on_update or []))
        orig_add(self, inst)

    def patched_drain(self, tick_clock, wait_clock):
        # reimplementation of the original: same drain -> barrier -> sem-clear
        # -> barrier sequence, but the drain's (many) waits are split into
        # standalone EventSemaphores emitted BEFORE the sem clear.
        from concourse.vector_clock import ScopedClock

        nc = self.nc
        drain_wrap = nc.sync.drain()
        drain_inst = drain_wrap.ins  # BassInstruction wrapper -> mybir inst
        wait_clock.add_sem_waits(
            drain_inst, ScopedClock({None: tick_clock.global_clock}))
        bb = nn(nc.cur_bb).bb
        si = getattr(drain_inst, "sync_info", None)
        if si is not None and si.on_wait and len(si.on_wait) > 1:
            waits = list(si.on_wait)
            drain_inst.sync_info = mybir.SyncInfo(
                on_wait=[waits[0]], on_update=list(si.on_update or []))
            for w in waits[1:]:
                nm = nc.get_next_instruction_name()
                ev = mybir.InstEventSemaphore(
                    name=nm, engine=drain_inst.engine, ins=[], outs=[],
                    sync_info=mybir.SyncInfo(on_wait=[w], on_update=[]))
                nc.register_instruction(ev, overwrite=True)
                bb.add_instruction(ev)

        nc.all_engine_barrier()
        assert self.sems is not None
        popped = nc._tile_sem_poison_stack.pop()
        assert popped is self._sem_poison
        nc.clear_and_free_semaphores(list(self.sems.allocated().values()))
        nc.all_engine_barrier()

    tile.TileContext._add_instruction = patched_add
    tile.TileContext._drain_and_barrier = patched_drain
    tile.TileContext._wait_split_installed = True


def _build_nc():
    import concourse.bass as bass
    import concourse.mybir as mybir
    import concourse.tile as tile

    _install_wait_splitter()

    f32 = mybir.dt.float32
    bf16 = mybir.dt.bfloat16

    nc = bass.Bass()

    qT = nc.declare_dram_parameter("qT", [IN_DIM, S], bf16, isOutput=False)
    wq = nc.declare_dram_parameter("wq", [IN_DIM, 128], bf16, isOutput=False)
    wk = nc.declare_dram_parameter("wk", [IN_DIM, 128], bf16, isOutput=False)
    wv = nc.declare_dram_parameter("wv", [IN_DIM, 128], bf16, isOutput=False)
    wo = nc.declare_dram_parameter("wo", [128, IN_DIM], bf16, isOutput=False)
    cc = nc.declare_dram_parameter("cc", [128, S], bf16, isOutput=False)
    ssp = nc.declare_dram_parameter("ssp", [128, S], bf16, isOutput=False)
    mb = nc.declare_dram_parameter("mb", [128, 4 * SC], bf16, isOutput=False)
    ident = nc.declare_dram_parameter("ident", [128, 128], bf16, isOutput=False)
    perm = nc.declare_dram_parameter("perm", [128, 128], bf16, isOutput=False)
    out = nc.declare_dram_parameter("out", [S, IN_DIM], bf16, isOutput=True)

    NSC = S // SC   # 4 s-chunks
    NAT = S // AT   # 16 a-tiles
    NIT = IN_DIM // 128  # 8 i-tiles
    EXP_SCALE = float(1.0 / np.sqrt(HEAD_DIM))

    with tile.TileContext(nc) as tc:
        with (
            tc.tile_pool(name="big", bufs=1) as big,
            tc.tile_pool(name="psum", bufs=4, space="PSUM") as psum,
            tc.tile_pool(name="tmp", bufs=3) as tmp,
            tc.tile_pool(name="attn", bufs=6) as attnp,
            tc.tile_pool(name="small", bufs=2) as small,
            tc.tile_pool(name="osb", bufs=2) as osb,
        ):
            # ---- resident SBUF tensors ----
            qT_sb = big.tile([128, NIT, S], bf16)
            wq_sb = big.tile([128, NIT, 128], bf16)
            wk_sb = big.tile([128, NIT, 128], bf16)
            wv_sb = big.tile([128, NIT, 128], bf16)
            wo_sb = big.tile([128, IN_DIM], bf16)
            cc_sb = big.tile([128, S], bf16)
            ssp_sb = big.tile([128, S], bf16)
            mb_sb = big.tile([128, 4 * SC], bf16)
            id_sb = big.tile([128, 128], bf16)
            pm_sb = big.tile([128, 128], bf16)
            qh_sb = big.tile([128, S], bf16)  # roped q, [2 heads x (32r|32i)], s
            kh_sb = big.tile([128, S], bf16)
            v_sb = big.tile([128, 2, NAT, HEAD_DIM + 1], bf16)  # [a, h, t, d+1]
            on_sb = big.tile([128, S], bf16)  # normalized outT, 2 heads stacked
            ones_sb = big.tile([1, HEAD_DIM], bf16)

            # Input DMA block: emission order = first-need order, transfers
            # split so the 16 queues deliver first-needed bytes in parallel
            # (a queue moves only ~20GB/s), and dma_start issue alternates
            # between the SP and Act sequencers (~350ns per issue each, both
            # idle during warmup) so issue serialization doesn't gate the
            # cold start.
            _iss = [0]

            def dma(out_ap, in_ap):
                eng = (nc.sync, nc.scalar)[_iss[0] % 2]
                _iss[0] += 1
                eng.dma_start(out=out_ap, in_=in_ap)

            def dma_qt_chunk(c):
                cs = slice(c * SC, (c + 1) * SC)
                for t in range(NIT):
                    dma(qT_sb[:, t, cs], qT[t * 128:(t + 1) * 128, cs])
                dma(cc_sb[:, cs], cc[:, cs])
                dma(ssp_sb[:, cs], ssp[:, cs])

            # warmup triples: (wq_t, wk_t, qT(0,t) in halves) in tile order,
            # tracking the interleaved q/k accumulation of chunk 0.
            h0 = SC // 2
            for t in range(NIT):
                dma(wq_sb[:, t, :], wq[t * 128:(t + 1) * 128, :])
                dma(wk_sb[:, t, :], wk[t * 128:(t + 1) * 128, :])
                dma(qT_sb[:, t, 0:h0], qT[t * 128:(t + 1) * 128, 0:h0])
                dma(qT_sb[:, t, h0:SC], qT[t * 128:(t + 1) * 128, h0:SC])
            dma(cc_sb[:, 0:SC], cc[:, 0:SC])
            dma(ssp_sb[:, 0:SC], ssp[:, 0:SC])
            for t in range(NIT):
                dma(wv_sb[:, t, :], wv[t * 128:(t + 1) * 128, :])
            dma(pm_sb, perm[:, :])
            dma_qt_chunk(1)
            for c in range(4):
                dma(mb_sb[:, c * SC:(c + 1) * SC], mb[:, c * SC:(c + 1) * SC])
            dma(id_sb, ident[:, :])
            dma_qt_chunk(2)
            dma(wo_sb[:, 0:SC], wo[:, 0:SC])
            dma(wo_sb[:, SC:IN_DIM], wo[:, SC:IN_DIM])
            dma_qt_chunk(3)
            nc.vector.memset(v_sb, 1.0)  # ones column for rowsums survives
            nc.vector.memset(ones_sb, 1.0)

            def emit_qk_finish_dve(j, accs):
                """rope partial products: t1 = x*cc (f32), t2p = x*ssP (bf16,
                feeds the partial-swap permutation matmul)."""
                cs = slice(j * SC, (j + 1) * SC)
                parts = []
                for ps_x in accs:
                    t2p = tmp.tile([128, SC], bf16, tag="t2p", name="t2p")
                    nc.vector.tensor_mul(t2p, ps_x, ssp_sb[:, cs])
                    t1 = tmp.tile([128, SC], f32, tag="t1", name="t1")
                    nc.vector.tensor_mul(t1, ps_x, cc_sb[:, cs])
                    parts.append((t1, t2p))
                return parts

            def emit_qk_finish_pe(j, parts):
                """swap halves of the ssP product via a permutation matmul,
                then combine: dst = x*cc + P(x*ssP)."""
                cs = slice(j * SC, (j + 1) * SC)
                for (t1, t2p), dst in zip(parts, (qh_sb, kh_sb)):
                    ps_t2 = psum.tile([128, SC], f32, tag="ps", name="ps_t2")
                    nc.tensor.matmul(ps_t2, pm_sb, t2p, start=True, stop=True)
                    nc.vector.tensor_add(dst[:, cs], t1, ps_t2)

            def emit_v_tile(t):
                ps_v = psum.tile([128, SC], f32, tag="ps", name="ps_v")
                for ti in range(NIT):
                    nc.tensor.matmul(
                        ps_v[:, 0:128],
                        qT_sb[:, ti, t * AT:(t + 1) * AT], wv_sb[:, ti, :],
                        start=(ti == 0), stop=(ti == NIT - 1))
                nc.vector.tensor_copy(
                    v_sb[:, :, t, 0:HEAD_DIM],
                    ps_v[:, 0:128].rearrange("p (h d) -> p h d", h=2))

            def emit_score(h, j, t, n_at):
                """scores matmul (+mask) for a-tile t of chunk j, head h,
                then exp into a bf16 attn tile. Returns (at, off)."""
                hp = slice(64 * h, 64 * h + 64)
                diag = t >= n_at - 4
                off = (t - 4 * j) * AT if diag else 0
                js = slice(j * SC + off, (j + 1) * SC)
                ps_s = psum.tile([128, SC], f32, tag="ps", name="ps_s")
                nc.tensor.matmul(
                    ps_s[:, off:SC],
                    kh_sb[hp, t * AT:(t + 1) * AT], qh_sb[hp, js],
                    start=True, stop=not diag)
                if diag:
                    c = t - 4 * j
                    nc.tensor.matmul(
                        ps_s[:, off:SC], id_sb,
                        mb_sb[:, c * SC + off:(c + 1) * SC],
                        start=False, stop=True)
                at = attnp.tile([128, SC], bf16, tag="at", name="at")
                nc.scalar.activation(
                    out=at[:, off:SC], in_=ps_s[:, off:SC],
                    func=mybir.ActivationFunctionType.Exp, scale=EXP_SCALE)
                return at, off

            def emit_attnv(h, t, n_at, ps_o, at, off):
                nc.tensor.matmul(
                    ps_o[0:HEAD_DIM + 1, off:SC],
                    v_sb[:, h, t, :], at[:, off:SC],
                    start=(t == 0), stop=(t == n_at - 1),
                    skip_group_check=True)

            def emit_rowsum(ps_o):
                """1/rowsum as exp(-ln(rowsum)), both on Act: no slow DVE
                reciprocal, and both funcs live in the natural_log_exp table
                so there is no activation-table reload."""
                ln_r = small.tile([1, SC], f32, tag="lnr", name="ln_r")
                nc.scalar.activation(
                    out=ln_r, in_=ps_o[HEAD_DIM:HEAD_DIM + 1, :],
                    func=mybir.ActivationFunctionType.Ln)
                rec_b = small.tile([1, SC], bf16, tag="recb", name="rec_b")
                nc.scalar.activation(
                    out=rec_b, in_=ln_r,
                    func=mybir.ActivationFunctionType.Exp, scale=-1.0)
                return rec_b

            def emit_normalize(h, j, ps_o, rec_b):
                hp = slice(64 * h, 64 * h + 64)
                js = slice(j * SC, (j + 1) * SC)
                ps_r = psum.tile([HEAD_DIM, SC], f32, tag="ps", name="ps_r")
                nc.tensor.matmul(ps_r, ones_sb, rec_b, start=True, stop=True)
                sb_r = tmp.tile([HEAD_DIM, SC], f32, tag="sbr", name="sb_r")
                nc.vector.tensor_copy(sb_r, ps_r)
                tn = tmp.tile([64, SC], bf16, tag="tn", name="tn")
                nc.vector.tensor_mul(tn, ps_o[0:HEAD_DIM, :], sb_r)
                nc.sync.dma_start(out=on_sb[hp, js], in_=tn)

            def emit_wo_m(m):
                # out DMAs split to 64KB quarters: the final m-tile's DMA is
                # on the critical tail, and one queue moves only ~20GB/s.
                o_sb = osb.tile([128, IN_DIM], bf16, tag="o", name="o_sb")
                for c in range(IN_DIM // SC):
                    ps = psum.tile([128, SC], f32, tag="ps", name="ps_w")
                    nc.tensor.matmul(
                        ps, on_sb[:, m * 128:(m + 1) * 128],
                        wo_sb[:, c * SC:(c + 1) * SC],
                        start=True, stop=True)
                    nc.vector.tensor_copy(o_sb[:, c * SC:(c + 1) * SC], ps)
                    for qq in range(2):
                        lo = c * SC + qq * (SC // 2)
                        nc.sync.dma_start(
                            out=out[m * 128:(m + 1) * 128, lo:lo + SC // 2],
                            in_=o_sb[:, lo:lo + SC // 2])

            def alloc_accs(i):
                return [psum.tile([128, SC], f32, tag="pa", bufs=2,
                                  name=f"ps_x{i}{k}") for k in range(2)]

            def emit_accum_half(j, accs, half):
                cs = slice(j * SC, (j + 1) * SC)
                for t in range(half * (NIT // 2), (half + 1) * (NIT // 2)):
                    for w_sb, ps_x in zip((wq_sb, wk_sb), accs):
                        nc.tensor.matmul(
                            ps_x, w_sb[:, t, :], qT_sb[:, t, cs],
                            start=(t == 0), stop=(t == NIT - 1))

            # ---- fused chunk loop, fully software-pipelined ----
            # Each attention iteration is Act-gated (2 exps ~ 1.15us vs
            # ~0.9us of its own matmuls), so every other chunk's PE work
            # (v proj, next-next chunk's qk accum, previous chunk's Wo) is
            # queued as filler thunks consumed one per iteration: the PE
            # stays continuously busy (holding its fast p-state) and all
            # cross-engine chains get covered by independent queued work.
            accs = {0: alloc_accs(0)}
            emit_accum_half(0, accs[0], 0)
            emit_accum_half(0, accs[0], 1)
            parts1 = emit_qk_finish_dve(0, accs[0])
            emit_qk_finish_pe(0, parts1)
            for t in range(4):
                emit_v_tile(t)
            accs[1] = alloc_accs(1)
            emit_accum_half(1, accs[1], 0)
            emit_accum_half(1, accs[1], 1)
            parts1 = None
            for j in range(NSC):
                n_at = 4 * (j + 1)
                queue = []
                if j > 0:
                    queue += [(emit_wo_m, (m,))
                              for m in range(4 * (j - 1), 4 * j)]
                vtail = []
                if j + 1 < NSC:
                    vt = list(range(4 * (j + 1), 4 * (j + 2)))
                    queue += [(emit_v_tile, (t,)) for t in vt[:2]]
                    vtail = vt[2:]
                if j + 2 < NSC:
                    accs[j + 2] = alloc_accs(j + 2)
                    queue += [(emit_accum_half, (j + 2, accs[j + 2], 0)),
                              (emit_accum_half, (j + 2, accs[j + 2], 1))]
                ps_o = [psum.tile([128, SC], f32, tag="po", bufs=2,
                                  name=f"ps_o{h}") for h in range(2)]
                pend = {}  # (h, t) -> (at, off)
                for h in range(2):
                    pend[(h, 0)] = emit_score(h, j, 0, n_at)
                for t in range(n_at):
                    if t + 1 < n_at:
                        for h in range(2):
                            pend[(h, t + 1)] = emit_score(h, j, t + 1, n_at)
                    for h in range(2):
                        at, off = pend.pop((h, t))
                        emit_attnv(h, t, n_at, ps_o[h], at, off)
                    if t == 0 and (j + 1) in accs:
                        parts1 = emit_qk_finish_dve(j + 1, accs[j + 1])
                    elif t == 2 and parts1 is not None:
                        emit_qk_finish_pe(j + 1, parts1)
                        parts1 = None
                    elif queue:
                        fn, args = queue.pop(0)
                        fn(*args)
                recs = [emit_rowsum(ps_o[h]) for h in range(2)]
                for fn, args in queue:
                    fn(*args)
                for t in vtail:
                    emit_v_tile(t)
                for h in range(2):
                    emit_normalize(h, j, ps_o[h], recs[h])
            for m in range(4 * (NSC - 1), 4 * NSC):
                emit_wo_m(m)

    return nc


def _host_prep(q, Wq, Wk, Wv, Wo):
    """Build the 8 per-core input maps (numpy, bf16 for matmul operands)."""
    import ml_dtypes
    bf16 = ml_dtypes.bfloat16

    deint = np.concatenate([np.arange(0, HEAD_DIM, 2),
                            np.arange(1, HEAD_DIM, 2)])  # de-interleave perm

    wq_eff = Wq.reshape(IN_DIM, KV_HEADS, GROUPS, HEAD_DIM).sum(axis=2)
    wq_d = wq_eff[:, :, deint]          # [1024, 4, 64] de-interleaved
    wk_d = Wk[:, :, deint]

    # rope tables, de-interleaved layout, one head-block repeated twice.
    # ssp is PRE-SWAPPED (rows [0:32]<->[32:64] within each 64-block): the
    # device computes P(x * ssp) with a permutation matmul, which equals
    # xswap * ss.
    pos = np.arange(1, S + 1, dtype=np.float64)
    thetas = 10.0 ** (-np.arange(HALF, dtype=np.float64))
    ang = pos[None, :] * thetas[:, None]          # [32, S]
    cosv, sinv = np.cos(ang), np.sin(ang)
    cc64 = np.concatenate([cosv, cosv], axis=0)   # [64, S]
    ssp64 = np.concatenate([sinv, -sinv], axis=0)  # P(ss), ss = [-sin|+sin]
    cc = np.concatenate([cc64, cc64], axis=0).astype(bf16)   # [128, S]
    ssp = np.concatenate([ssp64, ssp64], axis=0).astype(bf16)

    # half-swap permutation matrix (symmetric involution)
    r = np.arange(128)
    pr = (r // 64) * 64 + ((r % 64) + HALF) % 64
    perm = np.zeros((128, 128), dtype=np.float32)
    perm[pr, r] = 1.0
    perm = perm.astype(bf16)

    # additive causal bias for the 4 diagonal-crossing tiles, bf16
    i_idx = np.arange(AT)[:, None]
    j_idx = np.arange(SC)[None, :]
    mbs = []
    for c in range(4):
        delta = AT * c
        mbs.append(np.where(j_idx < i_idx + delta, MASK_BIAS, 0.0))
    mb = np.concatenate(mbs, axis=1).astype(bf16)  # [128, 2048]
    ident = np.eye(128, dtype=np.float32).astype(bf16)

    in_maps = []
    for core in range(N_CORES):
        b, hg = core // 2, core % 2
        heads = [2 * hg, 2 * hg + 1]
        wq_c = np.concatenate([wq_d[:, h, :] for h in heads], axis=1)
        wk_c = np.concatenate([wk_d[:, h, :] for h in heads], axis=1)
        wv_c = np.concatenate([Wv[:, h, :] for h in heads], axis=1)
        in_maps.append({
            "qT": np.ascontiguousarray(q[b].T).astype(bf16),
            "wq": wq_c.astype(bf16),
            "wk": wk_c.astype(bf16),
            "wv": wv_c.astype(bf16),
            "wo": Wo[hg * 128:(hg + 1) * 128, :].astype(bf16),
            "cc": cc, "ssp": ssp, "mb": mb, "ident": ident, "perm": perm,
        })
    return in_maps


def _install_ntff_hook():
    """Recreate the missing antenv.axon_hooks shim so trace=True works."""
    import sys, types
    if "antenv.axon_hooks" in sys.modules:
        return
    mod = types.ModuleType("antenv.axon_hooks")
    _hook = [None]
    mod.set_axon_ntff_profile_hook = lambda h: _hook.__setitem__(0, h)
    mod.get_axon_ntff_profile_hook = lambda: _hook[0]
    sys.modules["antenv.axon_hooks"] = mod
    try:
        if "/root/.axon_site" not in sys.path:
            sys.path.insert(0, "/root/.axon_site")
        from trn_agent_boot.trn_boot import _ntff_profile_via_ctypes
        mod.set_axon_ntff_profile_hook(
            _ntff_profile_via_ctypes("/opt/axon/libaxon_pjrt.so"))
    except Exception:
        pass


def kernel(q, mask, Wq, Wk, Wv, Wo, _trace=False):
    import sys
    if "/opt/trn_rl_repo" not in sys.path:
        sys.path.insert(0, "/opt/trn_rl_repo")
    if _trace:
        _install_ntff_hook()
    from concourse.bass_utils import run_bass_kernel_spmd

    if "nc" not in _cached:
        _cached["nc"] = _build_nc()
    nc = _cached["nc"]

    q = np.asarray(q, np.float32)
    in_maps = _host_prep(q, np.asarray(Wq, np.float32),
                         np.asarray(Wk, np.float32), np.asarray(Wv, np.float32),
                         np.asarray(Wo, np.float32))
    res = run_bass_kernel_spmd(nc, in_maps, core_ids=list(range(N_CORES)),
                               trace=_trace)
    parts = [np.asarray(r["out"], dtype=np.float32) for r in res.results]
    out = np.stack([parts[2 * b] + parts[2 * b + 1] for b in range(B)])
    if _trace:
        kernel.last_exec_time_ns = res.exec_time_ns
        kernel.last_results = res
    return out.astype(np.float32)


# revision 26
# speedup vs baseline: 1.1728x; 1.1160x over previous
"""GQA kernel for trn2, 8 NeuronCores.

Problem: nn_GroupedQueryAttention (b=4, s=2048, 16 q-heads / 4 kv-heads, d=64).
The reference's score einsum 'bghsd,bhad->bhsa' SUMS over the group axis g, and
RoPE is linear in x, so sum_g rope(q @ Wq[:,h*4+g,:]) == rope(q @ sum_g Wq).
The whole module therefore collapses to 4-head MHA with Wq pre-summed over
groups.

Sharding: 8 cores = (batch b in 0..3) x (head-group hg in 0..1, 2 heads each).
Each core computes its two heads' attention output and a partial product with
its 128-row slice of Wo; the host sums the two partials per batch.

Device layout choices (all host-prepped in numpy):
 - qT [1024, 2048] (q[b] transposed, bf16) so the contraction dim i sits on
   SBUF partitions for every projection matmul.
 - q/k projection weights are group-summed, and their 64 head-dims are
   de-interleaved to [32 real | 32 imag] so RoPE becomes
     roped = x * CC + xswap * SS
   where xswap is produced by a second matmul with column-swapped weights.
 - all matmul operands are bf16 (1 PE cycle/row vs 4 for fp32); PSUM
   accumulation stays fp32.
 - scores are computed TRANSPOSED (scoresT[a, s]) so no on-device transpose of
   the attention matrix is needed; softmax denominators come from a ones-column
   appended to V (the attn@V matmul computes them for free along M).
 - causal masking: additive -1e4 bias, injected into the scores PSUM
   accumulation group via an identity-matmul on the 4 diagonal-crossing tiles;
   score/exp/attnV streams on those tiles skip the fully-masked column prefix.
 - softmax 1/rowsum is broadcast across the 64 head dims with a tiny K=1
   PE matmul (ones[1,64] x rec[1,512]).

Scheduling: one fused loop over the 4 s-chunks. Chunk j emits
  qproj(j) -> kproj(j) -> vproj(tiles 4j..4j+3) -> attention(j)
and attention is software-pipelined two heads wide with one-tile lookahead
(score matmuls for tile t+1 are issued before attnV of tile t) so the PE
never head-of-line blocks on the Act engine's exp. The Wo output projection
for chunk j-1 is emitted inside chunk j's attention loop, overlapping the
output DMA with compute.
"""

import numpy as np

B, S, IN_DIM = 4, 2048, 1024
Q_HEADS, KV_HEADS, HEAD_DIM = 16, 4, 64
GROUPS = Q_HEADS // KV_HEADS
HALF = HEAD_DIM // 2  # 32
N_CORES = 8
SC = 512  # s-chunk width (psum bank)
AT = 128  # a-tile width
MASK_BIAS = -1.0e4

_cached = {}


def _install_wait_splitter():
    """This walrus build accepts only ONE semaphore wait per instruction.
    Tile emits several; hoist all-but-one into standalone EventSemaphores."""
    import concourse.mybir as mybir
    import concourse.tile as tile
    from concourse._compat import not_none as nn

    if getattr(tile.TileContext, "_wait_split_installed", False):
        return

    orig_add = tile.TileContext._add_instruction

    def patched_add(self, inst):
        si = getattr(inst, "sync_info", None)
        if si is not None and si.on_wait and len(si.on_wait) > 1:
            waits = list(si.on_wait)
            for w in waits[:-1]:
                nm = self.nc.get_next_instruction_name()
                ev = mybir.InstEventSemaphore(
                    name=nm, engine=inst.engine, ins=[], outs=[],
                    sync_info=mybir.SyncInfo(on_wait=[w], on_update=[]))
                orig_add(self, ev)
            inst.sync_info = mybir.SyncInfo(
                on_wait=[waits[-1]], on_update=list(si.on_update or []))
        orig_add(self, inst)

    def patched_drain(self, tick_clock, wait_clock):
        # reimplementation of the original: same drain -> barrier -> sem-clear
        # -> barrier sequence, but the drain's (many) waits are split into
        # standalone EventSemaphores emitted BEFORE the sem clear.
        from concourse.vector_clock import ScopedClock

        nc = self.nc
        drain_wrap = nc.sync.drain()
        drain_inst = drain_wrap.ins  # BassInstruction wrapper -> mybir inst
        wait_clock.add_sem_waits(
            drain_inst, ScopedClock({None: tick_clock.global_clock}))
        bb = nn(nc.cur_bb).bb
        si = getattr(drain_inst, "sync_info", None)
        if si is not None and si.on_wait and len(si.on_wait) > 1:
            waits = list(si.on_wait)
            drain_inst.sync_info = mybir.SyncInfo(
                on_wait=[waits[0]], on_update=list(si.on_update or []))
            for w in waits[1:]:
                nm = nc.get_next_instruction_name()
                ev = mybir.InstEventSemaphore(
                    name=nm, engine=drain_inst.engine, ins=[], outs=[],
                    sync_info=mybir.SyncInfo(on_wait=[w], on_update=[]))
                nc.register_instruction(ev, overwrite=True)
                bb.add_instruction(ev)

        nc.all_engine_barrier()
        assert self.sems is not None
        popped = nc._tile_sem_poison_stack.pop()
        assert popped is self._sem_poison
        nc.clear_and_free_semaphores(list(self.sems.allocated().values()))
        nc.all_engine_barrier()

    tile.TileContext._add_instruction = patched_add
    tile.TileContext._drain_and_barrier = patched_drain
    tile.TileContext._wait_split_installed = True


def _build_nc():
    import concourse.bass as bass
    import concourse.mybir as mybir
    import concourse.tile as tile

    _install_wait_splitter()

    f32 = mybir.dt.float32
    bf16 = mybir.dt.bfloat16

    nc = bass.Bass()

    qT = nc.declare_dram_parameter("qT", [IN_DIM, S], bf16, isOutput=False)
    wq = nc.declare_dram_parameter("wq", [IN_DIM, 128], bf16, isOutput=False)
    wk = nc.declare_dram_parameter("wk", [IN_DIM, 128], bf16, isOutput=False)
    wv = nc.declare_dram_parameter("wv", [IN_DIM, 128], bf16, isOutput=False)
    wo = nc.declare_dram_parameter("wo", [128, IN_DIM], bf16, isOutput=False)
    cc = nc.declare_dram_parameter("cc", [128, S], bf16, isOutput=False)
    ssp = nc.declare_dram_parameter("ssp", [128, S], bf16, isOutput=False)
    mb = nc.declare_dram_parameter("mb", [128, 4 * SC], bf16, isOutput=False)
    ident = nc.declare_dram_parameter("ident", [128, 128], bf16, isOutput=False)
    perm = nc.declare_dram_parameter("perm", [128, 128], bf16, isOutput=False)
    out = nc.declare_dram_parameter("out", [S, IN_DIM], bf16, isOutput=True)

    NSC = S // SC   # 4 s-chunks
    NAT = S // AT   # 16 a-tiles
    NIT = IN_DIM // 128  # 8 i-tiles
    EXP_SCALE = float(1.0 / np.sqrt(HEAD_DIM))

    with tile.TileContext(nc) as tc:
        with (
            tc.tile_pool(name="big", bufs=1) as big,
            tc.tile_pool(name="psum", bufs=4, space="PSUM") as psum,
            tc.tile_pool(name="tmp", bufs=3) as tmp,
            tc.tile_pool(name="attn", bufs=6) as attnp,
            tc.tile_pool(name="small", bufs=2) as small,
            tc.tile_pool(name="osb", bufs=2) as osb,
        ):
            # ---- resident SBUF tensors ----
            qT_sb = big.tile([128, NIT, S], bf16)
            wq_sb = big.tile([128, NIT, 128], bf16)
            wk_sb = big.tile([128, NIT, 128], bf16)
            wv_sb = big.tile([128, NIT, 128], bf16)
            wo_sb = big.tile([128, IN_DIM], bf16)
            cc_sb = big.tile([128, S], bf16)
            ssp_sb = big.tile([128, S], bf16)
            mb_sb = big.tile([128, 4 * SC], bf16)
            id_sb = big.tile([128, 128], bf16)
            pm_sb = big.tile([128, 128], bf16)
            qh_sb = big.tile([128, S], bf16)  # roped q, [2 heads x (32r|32i)], s
            kh_sb = big.tile([128, S], bf16)
            v_sb = big.tile([128, 2, NAT, HEAD_DIM + 1], bf16)  # [a, h, t, d+1]
            on_sb = big.tile([128, S], bf16)  # normalized outT, 2 heads stacked
            ones_sb = big.tile([1, HEAD_DIM], bf16)

            # Input DMA block: emission order = first-need order, transfers
            # split so the 16 queues deliver first-needed bytes in parallel
            # (a queue moves only ~20GB/s), and dma_start issue alternates
            # between the SP and Act sequencers (~350ns per issue each, both
            # idle during warmup) so issue serialization doesn't gate the
            # cold start.
            _iss = [0]

            def dma(out_ap, in_ap, alt=False):
                # alternate SP/Act issuers only for warmup-critical pieces;
                # everything else stays on SP so the Act queue is free for
                # exps once attention starts.
                eng = nc.sync
                if alt:
                    eng = (nc.sync, nc.scalar)[_iss[0] % 2]
                    _iss[0] += 1
                eng.dma_start(out=out_ap, in_=in_ap)

            def dma_qt_chunk(c):
                # prefetched chunks: 4 issues of 2 i-tiles each
                cs = slice(c * SC, (c + 1) * SC)
                for t in range(0, NIT, 2):
                    dma(qT_sb[:, t:t + 2, cs],
                        qT[t * 128:(t + 2) * 128, cs].rearrange(
                            "(t p) s -> p t s", p=128))
                dma(cc_sb[:, cs], cc[:, cs])
                dma(ssp_sb[:, cs], ssp[:, cs])

            # warmup: q/k weight halves, then chunk-0 qT per i-tile, in the
            # order the interleaved chunk-0 accumulation consumes them.
            for lo in (0, NIT // 2):
                for w_sb, w in ((wq_sb, wq), (wk_sb, wk)):
                    dma(w_sb[:, lo:lo + NIT // 2, :],
                        w[lo * 128:(lo + NIT // 2) * 128, :].rearrange(
                            "(t p) m -> p t m", p=128), alt=True)
                if lo == 0:
                    for t in range(NIT):
                        dma(qT_sb[:, t, 0:SC], qT[t * 128:(t + 1) * 128, 0:SC],
                            alt=True)
            dma(cc_sb[:, 0:SC], cc[:, 0:SC], alt=True)
            dma(ssp_sb[:, 0:SC], ssp[:, 0:SC], alt=True)
            for lo in (0, NIT // 2):
                dma(wv_sb[:, lo:lo + NIT // 2, :],
                    wv[lo * 128:(lo + NIT // 2) * 128, :].rearrange(
                        "(t p) m -> p t m", p=128), alt=True)
            dma(pm_sb, perm[:, :], alt=True)
            dma_qt_chunk(1)
            dma(mb_sb[:, 0:2 * SC], mb[:, 0:2 * SC])
            dma(mb_sb[:, 2 * SC:4 * SC], mb[:, 2 * SC:4 * SC])
            dma(id_sb, ident[:, :])
            dma_qt_chunk(2)
            dma(wo_sb[:, 0:SC], wo[:, 0:SC])
            dma(wo_sb[:, SC:IN_DIM], wo[:, SC:IN_DIM])
            dma_qt_chunk(3)
            nc.vector.memset(v_sb, 1.0)  # ones column for rowsums survives
            nc.vector.memset(ones_sb, 1.0)

            def emit_qk_finish_dve(j, accs):
                """rope partial products: t1 = x*cc (f32), t2p = x*ssP (bf16,
                feeds the partial-swap permutation matmul)."""
                cs = slice(j * SC, (j + 1) * SC)
                parts = []
                for ps_x in accs:
                    t2p = tmp.tile([128, SC], bf16, tag="t2p", name="t2p")
                    nc.vector.tensor_mul(t2p, ps_x, ssp_sb[:, cs])
                    t1 = tmp.tile([128, SC], f32, tag="t1", name="t1")
                    nc.vector.tensor_mul(t1, ps_x, cc_sb[:, cs])
                    parts.append((t1, t2p))
                return parts

            def emit_qk_finish_pe(j, parts):
                """swap halves of the ssP product via a permutation matmul,
                then combine: dst = x*cc + P(x*ssP)."""
                cs = slice(j * SC, (j + 1) * SC)
                for (t1, t2p), dst in zip(parts, (qh_sb, kh_sb)):
                    ps_t2 = psum.tile([128, SC], f32, tag="ps", name="ps_t2")
                    nc.tensor.matmul(ps_t2, pm_sb, t2p, start=True, stop=True)
                    nc.vector.tensor_add(dst[:, cs], t1, ps_t2)

            def emit_v_tile(t):
                ps_v = psum.tile([128, SC], f32, tag="ps", name="ps_v")
                for ti in range(NIT):
                    nc.tensor.matmul(
                        ps_v[:, 0:128],
                        qT_sb[:, ti, t * AT:(t + 1) * AT], wv_sb[:, ti, :],
                        start=(ti == 0), stop=(ti == NIT - 1))
                nc.vector.tensor_copy(
                    v_sb[:, :, t, 0:HEAD_DIM],
                    ps_v[:, 0:128].rearrange("p (h d) -> p h d", h=2))

            def emit_score(h, j, t, n_at):
                """scores matmul (+mask) for a-tile t of chunk j, head h,
                then exp into a bf16 attn tile. Returns (at, off)."""
                hp = slice(64 * h, 64 * h + 64)
                diag = t >= n_at - 4
                off = (t - 4 * j) * AT if diag else 0
                js = slice(j * SC + off, (j + 1) * SC)
                ps_s = psum.tile([128, SC], f32, tag="ps", name="ps_s")
                nc.tensor.matmul(
                    ps_s[:, off:SC],
                    kh_sb[hp, t * AT:(t + 1) * AT], qh_sb[hp, js],
                    start=True, stop=not diag)
                if diag:
                    c = t - 4 * j
                    nc.tensor.matmul(
                        ps_s[:, off:SC], id_sb,
                        mb_sb[:, c * SC + off:(c + 1) * SC],
                        start=False, stop=True)
                at = attnp.tile([128, SC], bf16, tag="at", name="at")
                nc.scalar.activation(
                    out=at[:, off:SC], in_=ps_s[:, off:SC],
                    func=mybir.ActivationFunctionType.Exp, scale=EXP_SCALE)
                return at, off

            def emit_attnv(h, t, n_at, ps_o, at, off):
                nc.tensor.matmul(
                    ps_o[0:HEAD_DIM + 1, off:SC],
                    v_sb[:, h, t, :], at[:, off:SC],
                    start=(t == 0), stop=(t == n_at - 1),
                    skip_group_check=True)

            def emit_rowsum(ps_o):
                """1/rowsum as exp(-ln(rowsum)), both on Act: no slow DVE
                reciprocal, and both funcs live in the natural_log_exp table
                so there is no activation-table reload."""
                ln_r = small.tile([1, SC], f32, tag="lnr", name="ln_r")
                nc.scalar.activation(
                    out=ln_r, in_=ps_o[HEAD_DIM:HEAD_DIM + 1, :],
                    func=mybir.ActivationFunctionType.Ln)
                rec_b = small.tile([1, SC], bf16, tag="recb", name="rec_b")
                nc.scalar.activation(
                    out=rec_b, in_=ln_r,
                    func=mybir.ActivationFunctionType.Exp, scale=-1.0)
                return rec_b

            def emit_normalize(h, j, ps_o, rec_b):
                hp = slice(64 * h, 64 * h + 64)
                js = slice(j * SC, (j + 1) * SC)
                ps_r = psum.tile([HEAD_DIM, SC], f32, tag="ps", name="ps_r")
                nc.tensor.matmul(ps_r, ones_sb, rec_b, start=True, stop=True)
                sb_r = tmp.tile([HEAD_DIM, SC], f32, tag="sbr", name="sb_r")
                nc.vector.tensor_copy(sb_r, ps_r)
                tn = tmp.tile([64, SC], bf16, tag="tn", name="tn")
                nc.vector.tensor_mul(tn, ps_o[0:HEAD_DIM, :], sb_r)
                nc.sync.dma_start(out=on_sb[hp, js], in_=tn)

            def emit_wo_m(m, tail=False):
                # out DMA per 512-col half, launched right after its copy;
                # the last chunk's issues go on the Act queue (idle by then)
                # to dodge the end-of-run SP-queue backlog.
                o_sb = osb.tile([128, IN_DIM], bf16, tag="o", name="o_sb")
                eng = nc.scalar if tail else nc.sync
                for c in range(IN_DIM // SC):
                    ps = psum.tile([128, SC], f32, tag="ps", name="ps_w")
                    nc.tensor.matmul(
                        ps, on_sb[:, m * 128:(m + 1) * 128],
                        wo_sb[:, c * SC:(c + 1) * SC],
                        start=True, stop=True)
                    nc.vector.tensor_copy(o_sb[:, c * SC:(c + 1) * SC], ps)
                    eng.dma_start(
                        out=out[m * 128:(m + 1) * 128, c * SC:(c + 1) * SC],
                        in_=o_sb[:, c * SC:(c + 1) * SC])

            def alloc_accs(i):
                return [psum.tile([128, SC], f32, tag="pa", bufs=2,
                                  name=f"ps_x{i}{k}") for k in range(2)]

            def emit_accum_half(j, accs, half):
                cs = slice(j * SC, (j + 1) * SC)
                for t in range(half * (NIT // 2), (half + 1) * (NIT // 2)):
                    for w_sb, ps_x in zip((wq_sb, wk_sb), accs):
                        nc.tensor.matmul(
                            ps_x, w_sb[:, t, :], qT_sb[:, t, cs],
                            start=(t == 0), stop=(t == NIT - 1))

            # ---- fused chunk loop, fully software-pipelined ----
            # Each attention iteration is Act-gated (2 exps ~ 1.15us vs
            # ~0.9us of its own matmuls), so every other chunk's PE work
            # (v proj, next-next chunk's qk accum, previous chunk's Wo) is
            # queued as filler thunks consumed one per iteration: the PE
            # stays continuously busy (holding its fast p-state) and all
            # cross-engine chains get covered by independent queued work.
            accs = {0: alloc_accs(0)}
            emit_accum_half(0, accs[0], 0)
            emit_accum_half(0, accs[0], 1)
            parts1 = emit_qk_finish_dve(0, accs[0])
            emit_qk_finish_pe(0, parts1)
            for t in range(4):
                emit_v_tile(t)
            accs[1] = alloc_accs(1)
            emit_accum_half(1, accs[1], 0)
            emit_accum_half(1, accs[1], 1)
            parts1 = None
            for j in range(NSC):
                n_at = 4 * (j + 1)
                queue = []
                if j > 0:
                    queue += [(emit_wo_m, (m,))
                              for m in range(4 * (j - 1), 4 * j)]
                vtail = []
                if j + 1 < NSC:
                    vt = list(range(4 * (j + 1), 4 * (j + 2)))
                    queue += [(emit_v_tile, (t,)) for t in vt[:2]]
                    vtail = vt[2:]
                if j + 2 < NSC:
                    accs[j + 2] = alloc_accs(j + 2)
                    queue += [(emit_accum_half, (j + 2, accs[j + 2], 0)),
                              (emit_accum_half, (j + 2, accs[j + 2], 1))]
                ps_o = [psum.tile([128, SC], f32, tag="po", bufs=2,
                                  name=f"ps_o{h}") for h in range(2)]
                pend = {}  # (h, t) -> (at, off)
                for h in range(2):
                    pend[(h, 0)] = emit_score(h, j, 0, n_at)
                for t in range(n_at):
                    if t + 1 < n_at:
                        for h in range(2):
                            pend[(h, t + 1)] = emit_score(h, j, t + 1, n_at)
                    for h in range(2):
                        at, off = pend.pop((h, t))
                        emit_attnv(h, t, n_at, ps_o[h], at, off)
                    if t == 0 and (j + 1) in accs:
                        parts1 = emit_qk_finish_dve(j + 1, accs[j + 1])
                    elif t == 2 and parts1 is not None:
                        emit_qk_finish_pe(j + 1, parts1)
                        parts1 = None
                    elif queue:
                        fn, args = queue.pop(0)
                        fn(*args)
                recs = [emit_rowsum(ps_o[h]) for h in range(2)]
                for fn, args in queue:
                    fn(*args)
                for t in vtail:
                    emit_v_tile(t)
                for h in range(2):
                    emit_normalize(h, j, ps_o[h], recs[h])
            for m in range(4 * (NSC - 1), 4 * NSC):
                emit_wo_m(m, tail=True)

    return nc


def _host_prep(q, Wq, Wk, Wv, Wo):
    """Build the 8 per-core input maps (numpy, bf16 for matmul operands)."""
    import ml_dtypes
    bf16 = ml_dtypes.bfloat16

    deint = np.concatenate([np.arange(0, HEAD_DIM, 2),
                            np.arange(1, HEAD_DIM, 2)])  # de-interleave perm

    wq_eff = Wq.reshape(IN_DIM, KV_HEADS, GROUPS, HEAD_DIM).sum(axis=2)
    wq_d = wq_eff[:, :, deint]          # [1024, 4, 64] de-interleaved
    wk_d = Wk[:, :, deint]

    # rope tables, de-interleaved layout, one head-block repeated twice.
    # ssp is PRE-SWAPPED (rows [0:32]<->[32:64] within each 64-block): the
    # device computes P(x * ssp) with a permutation matmul, which equals
    # xswap * ss.
    pos = np.arange(1, S + 1, dtype=np.float64)
    thetas = 10.0 ** (-np.arange(HALF, dtype=np.float64))
    ang = pos[None, :] * thetas[:, None]          # [32, S]
    cosv, sinv = np.cos(ang), np.sin(ang)
    cc64 = np.concatenate([cosv, cosv], axis=0)   # [64, S]
    ssp64 = np.concatenate([sinv, -sinv], axis=0)  # P(ss), ss = [-sin|+sin]
    cc = np.concatenate([cc64, cc64], axis=0).astype(bf16)   # [128, S]
    ssp = np.concatenate([ssp64, ssp64], axis=0).astype(bf16)

    # half-swap permutation matrix (symmetric involution)
    r = np.arange(128)
    pr = (r // 64) * 64 + ((r % 64) + HALF) % 64
    perm = np.zeros((128, 128), dtype=np.float32)
    perm[pr, r] = 1.0
    perm = perm.astype(bf16)

    # additive causal bias for the 4 diagonal-crossing tiles, bf16
    i_idx = np.arange(AT)[:, None]
    j_idx = np.arange(SC)[None, :]
    mbs = []
    for c in range(4):
        delta = AT * c
        mbs.append(np.where(j_idx < i_idx + delta, MASK_BIAS, 0.0))
    mb = np.concatenate(mbs, axis=1).astype(bf16)  # [128, 2048]
    ident = np.eye(128, dtype=np.float32).astype(bf16)

    in_maps = []
    for core in range(N_CORES):
        b, hg = core // 2, core % 2
        heads = [2 * hg, 2 * hg + 1]
        wq_c = np.concatenate([wq_d[:, h, :] for h in heads], axis=1)
        wk_c = np.concatenate([wk_d[:, h, :] for h in heads], axis=1)
        wv_c = np.concatenate([Wv[:, h, :] for h in heads], axis=1)
        in_maps.append({
            "qT": np.ascontiguousarray(q[b].T).astype(bf16),
            "wq": wq_c.astype(bf16),
            "wk": wk_c.astype(bf16),
            "wv": wv_c.astype(bf16),
            "wo": Wo[hg * 128:(hg + 1) * 128, :].astype(bf16),
            "cc": cc, "ssp": ssp, "mb": mb, "ident": ident, "perm": perm,
        })
    return in_maps


def _install_ntff_hook():
    """Recreate the missing antenv.axon_hooks shim so trace=True works."""
    import sys, types
    if "antenv.axon_hooks" in sys.modules:
        return
    mod = types.ModuleType("antenv.axon_hooks")
    _hook = [None]
    mod.set_axon_ntff_profile_hook = lambda h: _hook.__setitem__(0, h)
    mod.get_axon_ntff_profile_hook = lambda: _hook[0]
    sys.modules["antenv.axon_hooks"] = mod
    try:
        if "/root/.axon_site" not in sys.path:
            sys.path.insert(0, "/root/.axon_site")
        from trn_agent_boot.trn_boot import _ntff_profile_via_ctypes
        mod.set_axon_ntff_profile_hook(
            _ntff_profile_via_ctypes("/opt/axon/libaxon_pjrt.so"))
    except Exception:
        pass


def kernel(q, mask, Wq, Wk, Wv, Wo, _trace=False):
    import sys
    if "/opt/trn_rl_repo" not in sys.path:
        sys.path.insert(0, "/opt/trn_rl_repo")
    if _trace:
        _install_ntff_hook()
    from concourse.bass_utils import run_bass_kernel_spmd

    if "nc" not in _cached:
        _cached["nc"] = _build_nc()
    nc = _cached["nc"]

    q = np.asarray(q, np.float32)
    in_maps = _host_prep(q, np.asarray(Wq, np.float32),
                         np.asarray(Wk, np.float32), np.asarray(Wv, np.float32),
                         np.asarray(Wo, np.float32))
    res = run_bass_kernel_spmd(nc, in_maps, core_ids=list(range(N_CORES)),
                               trace=_trace)
    parts = [np.asarray(r["out"], dtype=np.float32) for r in res.results]
    out = np.stack([parts[2 * b] + parts[2 * b + 1] for b in range(B)])
    if _trace:
        kernel.last_exec_time_ns = res.exec_time_ns
        kernel.last_results = res
    return out.astype(np.float32)
